# revision 9
# baseline (speedup 1.0000x reference)
"""Trainium2 Bass kernel: single-step attention GRU decoder (AttnDecoderRNN).

Contract: kernel(**inputs) takes the FULL unsharded inputs (as produced by the
problem's setup_inputs) and returns the FULL outputs
(log_softmax_output [64,50257], h_new [1,64,1024], attn_weights [64,128]).

Strategy (8 NeuronCores, SPMD — one program, per-core input shards):
  - embedding gather on host (64 rows of the 206MB table); the table itself is
    never shipped to the device.
  - stage A (attention logits + softmax): batch-sharded, 8 batches/core.
  - stage B (context = attn @ enc): batch-sharded; AllGather -> full ctx.
  - stage C (attn_combine + relu): output-column-sharded (128 cols/core);
    result transposed on-chip and AllGather'd so every core holds x^T.
  - stage D (GRU cell): hidden-unit-sharded (128 units/core, the matching
    r/z/n rows of w_ih/w_hh); AllGather of h_new^T.
  - stage E (vocab projection + log_softmax): vocab-sharded — each core streams
    its 25.7MB slice of out_w^T (the dominant memory traffic) and computes
    logits for 6283 vocab columns; a tiny AllGather of per-core (max, sumexp)
    stats produces the exact global log-softmax normalizer.
  All weight matrices are pre-transposed on the host so the contraction dim
  lands on SBUF partitions; biases are folded in via K=1 matmuls against a
  ones row (so the kernel stays correct for nonzero biases).
"""

import sys

for _p in ("/opt/trn_rl_repo",):
    if _p not in sys.path:
        sys.path.insert(0, _p)

import numpy as np

import concourse.bacc as bacc
import concourse.bass as bass
import concourse.mybir as mybir
import concourse.tile as tile
from concourse.bass_utils import run_bass_kernel_spmd
from concourse.masks import make_identity

F32 = mybir.dt.float32
AX = mybir.AxisListType.X

NCORES = 8
B = 64                  # batch
BS = B // NCORES        # batch shard (8)
L = 128                 # max src length
D = 1024                # emb = decoder hidden = encoder hidden
KT = D // 128           # 128-row contraction tiles over D
ES = D // NCORES        # hidden / combine output shard width (128)
G3 = 3 * ES             # GRU gate-shard width (384)
V = 50257
VS = 6283               # vocab shard width (8 * 6283 = 50264 >= V)
VP = NCORES * VS
VCH = 1024              # logits psum chunk width
NCH = (VS + VCH - 1) // VCH   # 7 (6x1024 + 139)
NEG_BIG = -1.0e30
WBUFS = 16              # out_w^T stream tiles in flight ([128,1024] f32 each)

_PROGRAM_CACHE = {}


def _build_program():
    nc = bacc.Bacc(
        "TRN2", target_bir_lowering=False, debug=False, num_devices=NCORES
    )

    def din(name, shape):
        return nc.dram_tensor(name, list(shape), F32, kind="ExternalInput")

    def dout(name, shape):
        return nc.dram_tensor(name, list(shape), F32, kind="ExternalOutput")

    # --- per-core / replicated inputs ---
    embT_sl = din("embT_sl", [D, BS])        # embedded^T, this core's batches
    h0T_sl = din("h0T_sl", [D, BS])
    embT_full = din("embT_full", [D, B])     # replicated
    h0T_full = din("h0T_full", [D, B])       # replicated
    h0_sl = din("h0_sl", [B, ES])            # h0 cols for this core's units
    enc_sl = din("enc_sl", [L, BS, D])       # encoder outputs, batch shard
    attn_wT = din("attn_wT", [2 * D, L])     # replicated
    attn_b = din("attn_b", [1, L])
    comb_wT_sl = din("comb_wT_sl", [2 * D, ES])
    comb_b_sl = din("comb_b_sl", [1, ES])
    w_ihT_sl = din("w_ihT_sl", [D, G3])
    b_ih_sl = din("b_ih_sl", [1, G3])
    w_hhT_sl = din("w_hhT_sl", [D, G3])
    b_hh_sl = din("b_hh_sl", [1, G3])
    out_wT_sl = din("out_wT_sl", [D, VS])
    out_b_sl = din("out_b_sl", [1, VS])

    # --- outputs ---
    out_logp = dout("out_logp", [B, VS])
    out_hnew = dout("out_hnew", [B, ES])
    out_attn = dout("out_attn", [B, L])

    # --- collective bounce buffers (internal DRAM) ---
    cc_attn_in = nc.dram_tensor("cc_attn_in", [BS, L], F32)
    cc_attn_out = nc.dram_tensor("cc_attn_out", [B, L], F32, addr_space="Shared")
    cc_ctx_in = nc.dram_tensor("cc_ctx_in", [BS, D], F32)
    cc_ctx_out = nc.dram_tensor("cc_ctx_out", [B, D], F32, addr_space="Shared")
    cc_x_in = nc.dram_tensor("cc_x_in", [ES, B], F32)
    cc_x_out = nc.dram_tensor("cc_x_out", [D, B], F32, addr_space="Shared")
    cc_h_in = nc.dram_tensor("cc_h_in", [ES, B], F32)
    cc_h_out = nc.dram_tensor("cc_h_out", [D, B], F32, addr_space="Shared")
    cc_st_in = nc.dram_tensor("cc_st_in", [B, 2], F32)
    cc_st_out = nc.dram_tensor("cc_st_out", [NCORES * B, 2], F32, addr_space="Shared")

    RG = [list(range(NCORES))]

    with tile.TileContext(nc) as tc:
        with (
            tc.tile_pool(name="const", bufs=1) as cp,
            tc.tile_pool(name="work", bufs=1) as wk,
            tc.tile_pool(name="pipe", bufs=2) as pipe,
            tc.tile_pool(name="encp", bufs=1) as ep,
            tc.tile_pool(name="wstream", bufs=WBUFS) as wp,
            tc.tile_pool(name="obias", bufs=2) as obp,
            tc.tile_pool(name="psum", bufs=2, space="PSUM") as pp,
        ):
            # ---------- constants ----------
            ident64 = cp.tile([B, B], F32)
            make_identity(nc, ident64[:])
            ident8 = cp.tile([BS, BS], F32)
            make_identity(nc, ident8[:])
            ones = cp.tile([1, B], F32)
            nc.gpsimd.memset(ones[:], 1.0)

            # ---------- small input loads ----------
            embT_sl_sb = cp.tile([128, KT, BS], F32)
            nc.sync.dma_start(embT_sl_sb[:], embT_sl[:].rearrange("(t p) b -> p t b", p=128))
            h0T_sl_sb = cp.tile([128, KT, BS], F32)
            nc.sync.dma_start(h0T_sl_sb[:], h0T_sl[:].rearrange("(t p) b -> p t b", p=128))
            embT_full_sb = cp.tile([128, KT, B], F32)
            nc.sync.dma_start(embT_full_sb[:], embT_full[:].rearrange("(t p) b -> p t b", p=128))
            h0T_full_sb = cp.tile([128, KT, B], F32)
            nc.sync.dma_start(h0T_full_sb[:], h0T_full[:].rearrange("(t p) b -> p t b", p=128))
            h0_sl_sb = cp.tile([B, ES], F32)
            nc.sync.dma_start(h0_sl_sb[:], h0_sl[:])
            attn_wT_sb = cp.tile([128, 2 * KT, L], F32, tag="w2k")
            nc.sync.dma_start(attn_wT_sb[:], attn_wT[:].rearrange("(t p) l -> p t l", p=128))
            attn_b_sb = cp.tile([1, L], F32)
            nc.sync.dma_start(attn_b_sb[:], attn_b[:])
            comb_wT_sb = cp.tile([128, 2 * KT, ES], F32, tag="w2k")
            nc.sync.dma_start(comb_wT_sb[:], comb_wT_sl[:].rearrange("(t p) e -> p t e", p=128))
            comb_b_sb = cp.tile([1, ES], F32)
            nc.sync.dma_start(comb_b_sb[:], comb_b_sl[:])
            w_ihT_sb = cp.tile([128, KT, G3], F32)
            nc.sync.dma_start(w_ihT_sb[:], w_ihT_sl[:].rearrange("(t p) g -> p t g", p=128))
            b_ih_sb = cp.tile([1, G3], F32)
            nc.sync.dma_start(b_ih_sb[:], b_ih_sl[:])
            w_hhT_sb = cp.tile([128, KT, G3], F32)
            nc.sync.dma_start(w_hhT_sb[:], w_hhT_sl[:].rearrange("(t p) g -> p t g", p=128))
            b_hh_sb = cp.tile([1, G3], F32)
            nc.sync.dma_start(b_hh_sb[:], b_hh_sl[:])

            # ---------- stage A: attention softmax (this core's 8 batches) ----
            ps_a = pp.tile([BS, L], F32, tag="small")
            for t in range(2 * KT):
                lhsT = embT_sl_sb[:, t, :] if t < KT else h0T_sl_sb[:, t - KT, :]
                nc.tensor.matmul(
                    ps_a[:], lhsT, attn_wT_sb[:, t, :],
                    start=(t == 0), stop=False,
                )
            nc.tensor.matmul(
                ps_a[:], ones[:1, :BS], attn_b_sb[:1, :], start=False, stop=True
            )
            amax = wk.tile([BS, 1], F32, tag="st1")
            nc.vector.reduce_max(amax[:], ps_a[:], AX)
            namax = wk.tile([BS, 1], F32, tag="st2")
            nc.vector.tensor_scalar_mul(namax[:], amax[:], -1.0)
            aexp = wk.tile([BS, L], F32, tag="aexp")
            asum = wk.tile([BS, 1], F32, tag="st3")
            nc.scalar.activation(
                aexp[:], ps_a[:], mybir.ActivationFunctionType.Exp,
                bias=namax[:], scale=1.0, accum_out=asum[:],
            )
            arec = wk.tile([BS, 1], F32, tag="st4")
            nc.vector.reciprocal(arec[:], asum[:])
            attn_sm = wk.tile([BS, L], F32, tag="attn_sm")
            nc.vector.tensor_scalar_mul(attn_sm[:], aexp[:], arec[:])

            # full attn weights (output #3) via AllGather; off critical path
            nc.sync.dma_start(cc_attn_in[:], attn_sm[:])
            nc.gpsimd.collective_compute(
                "AllGather", mybir.AluOpType.bypass,
                replica_groups=RG, ins=[cc_attn_in[:]], outs=[cc_attn_out[:]],
            )
            nc.sync.dma_start(out_attn[:], cc_attn_out[:])

            # attn^T for stage B
            ps_at = pp.tile([L, BS], F32, tag="small")
            nc.tensor.transpose(ps_at[:], attn_sm[:], ident8[:])
            attnT_sb = cp.tile([L, BS], F32)
            nc.vector.tensor_copy(attnT_sb[:], ps_at[:])

            # ---------- stage B: ctx for this core's batches ----------
            # PE output and compute-engine SBUF accesses must start at
            # partition 0/32/64/96, so each batch's ctx matvec lives in its own
            # [1, D] row at partition 0 and is DMA'd into its cc_ctx_in row.
            for h in range(2):
                et = ep.tile([L, BS // 2, D], F32, tag="enc")
                nc.sync.dma_start(et[:], enc_sl[:, 4 * h:4 * h + 4, :])
                for b in range(BS // 2):
                    lb = 4 * h + b
                    ps_cb = pp.tile([1, D], F32, tag="small")
                    for c0 in range(0, D, 512):
                        nc.tensor.matmul(
                            ps_cb[:, c0:c0 + 512],
                            attnT_sb[:, lb:lb + 1],
                            et[:, b, c0:c0 + 512],
                            start=True, stop=True, skip_group_check=True,
                        )
                    crow = pipe.tile([1, D], F32, tag="crow")
                    if lb % 2 == 0:
                        nc.vector.tensor_copy(crow[:], ps_cb[:])
                    else:
                        nc.scalar.copy(crow[:], ps_cb[:])
                    nc.sync.dma_start(cc_ctx_in[lb:lb + 1, :], crow[:])
            nc.gpsimd.collective_compute(
                "AllGather", mybir.AluOpType.bypass,
                replica_groups=RG, ins=[cc_ctx_in[:]], outs=[cc_ctx_out[:]],
            )
            ctx_sb = wk.tile([B, D], F32, tag="ctx_sb")
            nc.sync.dma_start(ctx_sb[:], cc_ctx_out[:])
            ctxT_sb = cp.tile([128, KT, B], F32)
            for i in range(KT):
                ps_tr = pp.tile([128, B], F32, tag="small")
                nc.tensor.transpose(ps_tr[:], ctx_sb[:, 128 * i:128 * (i + 1)], ident64[:])
                nc.vector.tensor_copy(ctxT_sb[:, i, :], ps_tr[:])

            # ---------- stage C: x = relu([emb, ctx] @ comb_w^T + b), col shard
            ps_x = pp.tile([B, ES], F32, tag="small")
            for t in range(2 * KT):
                lhsT = embT_full_sb[:, t, :] if t < KT else ctxT_sb[:, t - KT, :]
                nc.tensor.matmul(
                    ps_x[:], lhsT, comb_wT_sb[:, t, :], start=(t == 0), stop=False
                )
            nc.tensor.matmul(
                ps_x[:], ones[:1, :B], comb_b_sb[:1, :], start=False, stop=True
            )
            x_sb = wk.tile([B, ES], F32, tag="x_sb")
            nc.scalar.activation(x_sb[:], ps_x[:], mybir.ActivationFunctionType.Relu)
            ps_xt = pp.tile([ES, B], F32, tag="small")
            nc.tensor.transpose(ps_xt[:], x_sb[:], ident64[:])
            xT_sl_sb = wk.tile([ES, B], F32, tag="xT_sl")
            nc.vector.tensor_copy(xT_sl_sb[:], ps_xt[:])
            nc.sync.dma_start(cc_x_in[:], xT_sl_sb[:])
            nc.gpsimd.collective_compute(
                "AllGather", mybir.AluOpType.bypass,
                replica_groups=RG, ins=[cc_x_in[:]], outs=[cc_x_out[:]],
            )
            xT_sb = cp.tile([128, KT, B], F32)
            nc.sync.dma_start(xT_sb[:], cc_x_out[:].rearrange("(t p) b -> p t b", p=128))

            # ---------- stage D: GRU, hidden-unit shard ----------
            ps_gi = pp.tile([B, G3], F32, tag="small")
            for t in range(KT):
                nc.tensor.matmul(
                    ps_gi[:], xT_sb[:, t, :], w_ihT_sb[:, t, :],
                    start=(t == 0), stop=False,
                )
            nc.tensor.matmul(
                ps_gi[:], ones[:1, :B], b_ih_sb[:1, :], start=False, stop=True
            )
            ps_gh = pp.tile([B, G3], F32, tag="small")
            for t in range(KT):
                nc.tensor.matmul(
                    ps_gh[:], h0T_full_sb[:, t, :], w_hhT_sb[:, t, :],
                    start=(t == 0), stop=False,
                )
            nc.tensor.matmul(
                ps_gh[:], ones[:1, :B], b_hh_sb[:1, :], start=False, stop=True
            )
            # gates: layout [r | z | n], each ES wide.
            # HW DVE ops may read only one non-scalar PSUM input, so move gh
            # to SBUF first.
            gh_sb = wk.tile([B, G3], F32, tag="gh_sb")
            nc.vector.tensor_copy(gh_sb[:], ps_gh[:])
            rz = wk.tile([B, 2 * ES], F32, tag="rz")
            nc.vector.tensor_add(rz[:], ps_gi[:, :2 * ES], gh_sb[:, :2 * ES])
            r_sb = wk.tile([B, ES], F32, tag="r_sb")
            nc.scalar.activation(r_sb[:], rz[:, :ES], mybir.ActivationFunctionType.Sigmoid)
            z_sb = wk.tile([B, ES], F32, tag="z_sb")
            nc.scalar.activation(z_sb[:], rz[:, ES:], mybir.ActivationFunctionType.Sigmoid)
            rhn = wk.tile([B, ES], F32, tag="rhn")
            nc.vector.tensor_mul(rhn[:], r_sb[:], gh_sb[:, 2 * ES:])
            npre = wk.tile([B, ES], F32, tag="npre")
            nc.vector.tensor_add(npre[:], ps_gi[:, 2 * ES:], rhn[:])
            n_sb = wk.tile([B, ES], F32, tag="n_sb")
            nc.scalar.activation(n_sb[:], npre[:], mybir.ActivationFunctionType.Tanh)
            # h_new = n + z * (h0 - n)
            d1 = wk.tile([B, ES], F32, tag="d1")
            nc.vector.tensor_sub(d1[:], h0_sl_sb[:], n_sb[:])
            d2 = wk.tile([B, ES], F32, tag="d2")
            nc.vector.tensor_mul(d2[:], d1[:], z_sb[:])
            h_sb = wk.tile([B, ES], F32, tag="h_sb")
            nc.vector.tensor_add(h_sb[:], n_sb[:], d2[:])
            nc.sync.dma_start(out_hnew[:], h_sb[:])
            ps_ht = pp.tile([ES, B], F32, tag="small")
            nc.tensor.transpose(ps_ht[:], h_sb[:], ident64[:])
            hT_sl_sb = wk.tile([ES, B], F32, tag="hT_sl")
            nc.vector.tensor_copy(hT_sl_sb[:], ps_ht[:])
            nc.sync.dma_start(cc_h_in[:], hT_sl_sb[:])
            nc.gpsimd.collective_compute(
                "AllGather", mybir.AluOpType.bypass,
                replica_groups=RG, ins=[cc_h_in[:]], outs=[cc_h_out[:]],
            )
            hT_sb = cp.tile([128, KT, B], F32)
            nc.sync.dma_start(hT_sb[:], cc_h_out[:].rearrange("(t p) b -> p t b", p=128))

            # ---------- stage E: vocab-shard logits + log_softmax ----------
            logits_sb = cp.tile([B, VS], F32)
            cmax_sb = cp.tile([B, NCH], F32)
            for c in range(NCH):
                w0 = c * VCH
                cw = min(VCH, VS - w0)
                obt = obp.tile([1, VCH], F32, tag="ob")
                nc.sync.dma_start(obt[:1, :cw], out_b_sl[:, w0:w0 + cw])
                wts = []
                for k in range(KT):
                    wt = wp.tile([128, VCH], F32, tag="wt")
                    nc.sync.dma_start(
                        wt[:, :cw], out_wT_sl[128 * k:128 * (k + 1), w0:w0 + cw]
                    )
                    wts.append(wt)
                ps_l = pp.tile([B, VCH], F32, tag="big")
                for s0 in range(0, cw, 512):
                    sw = min(512, cw - s0)
                    for k in range(KT):
                        nc.tensor.matmul(
                            ps_l[:, s0:s0 + sw], hT_sb[:, k, :], wts[k][:, s0:s0 + sw],
                            start=(k == 0), stop=False, skip_group_check=True,
                        )
                    nc.tensor.matmul(
                        ps_l[:, s0:s0 + sw], ones[:1, :B], obt[:1, s0:s0 + sw],
                        start=False, stop=True, skip_group_check=True,
                    )
                nc.vector.tensor_copy(logits_sb[:, w0:w0 + cw], ps_l[:, :cw])
                nc.vector.reduce_max(cmax_sb[:, c:c + 1], ps_l[:, :cw], AX)

            # local stats: max and sum(exp(logit - local max))
            lmax = wk.tile([B, 1], F32, tag="lmax")
            nc.vector.reduce_max(lmax[:], cmax_sb[:], AX)
            nlmax = wk.tile([B, 1], F32, tag="nlmax")
            nc.vector.tensor_scalar_mul(nlmax[:], lmax[:], -1.0)
            sums_sb = wk.tile([B, NCH], F32, tag="sums")
            for c in range(NCH):
                w0 = c * VCH
                cw = min(VCH, VS - w0)
                ex = pipe.tile([B, VCH], F32, tag="exps")
                nc.scalar.activation(
                    ex[:, :cw], logits_sb[:, w0:w0 + cw],
                    mybir.ActivationFunctionType.Exp,
                    bias=nlmax[:], scale=1.0, accum_out=sums_sb[:, c:c + 1],
                )
            lsum = wk.tile([B, 1], F32, tag="lsum")
            nc.vector.reduce_sum(lsum[:], sums_sb[:], AX)
            st_sb = wk.tile([B, 2], F32, tag="st")
            nc.vector.tensor_copy(st_sb[:, 0:1], lmax[:])
            nc.vector.tensor_copy(st_sb[:, 1:2], lsum[:])
            nc.sync.dma_start(cc_st_in[:], st_sb[:])
            nc.gpsimd.collective_compute(
                "AllGather", mybir.AluOpType.bypass,
                replica_groups=RG, ins=[cc_st_in[:]], outs=[cc_st_out[:]],
            )
            stg = wk.tile([B, NCORES, 2], F32, tag="stg")
            nc.sync.dma_start(stg[:], cc_st_out[:].rearrange("(i b) s -> b i s", i=NCORES))
            gmax = wk.tile([B, 1], F32, tag="gmax")
            nc.vector.reduce_max(gmax[:], stg[:, :, 0], AX)
            ngmax = wk.tile([B, 1], F32, tag="ngmax")
            nc.vector.tensor_scalar_mul(ngmax[:], gmax[:], -1.0)
            # total = sum_i sumexp_i * exp(max_i - gmax)
            t1 = wk.tile([B, NCORES], F32, tag="t1")
            nc.scalar.activation(
                t1[:], stg[:, :, 0], mybir.ActivationFunctionType.Exp,
                bias=ngmax[:], scale=1.0,
            )
            t2 = wk.tile([B, NCORES], F32, tag="t2")
            nc.vector.tensor_mul(t2[:], t1[:], stg[:, :, 1])
            tot = wk.tile([B, 1], F32, tag="tot")
            nc.vector.reduce_sum(tot[:], t2[:], AX)
            lnt = wk.tile([B, 1], F32, tag="lnt")
            nc.scalar.activation(lnt[:], tot[:], mybir.ActivationFunctionType.Ln)
            norm = wk.tile([B, 1], F32, tag="norm")
            nc.vector.tensor_add(norm[:], gmax[:], lnt[:])
            nnorm = wk.tile([B, 1], F32, tag="nnorm")
            nc.vector.tensor_scalar_mul(nnorm[:], norm[:], -1.0)
            # final: out = logits - (gmax + ln total)
            for c in range(NCH):
                w0 = c * VCH
                cw = min(VCH, VS - w0)
                oc = pipe.tile([B, VCH], F32, tag="outc")
                nc.scalar.activation(
                    oc[:, :cw], logits_sb[:, w0:w0 + cw],
                    mybir.ActivationFunctionType.Identity,
                    bias=nnorm[:], scale=1.0,
                )
                nc.sync.dma_start(out_logp[:, w0:w0 + cw], oc[:, :cw])

    nc.compile()
    return nc


def _get_program():
    if "nc" not in _PROGRAM_CACHE:
        _PROGRAM_CACHE["nc"] = _build_program()
    return _PROGRAM_CACHE["nc"]


def _f32(a):
    return np.ascontiguousarray(np.asarray(a), dtype=np.float32)


def _prep_in_maps(inputs):
    idx = np.asarray(inputs["input_tensor"]).reshape(-1).astype(np.int64)
    assert idx.shape == (B,)
    emb_table = np.asarray(inputs["emb_table"])
    embedded = _f32(emb_table[idx])                    # [B, D] host gather
    h0 = _f32(np.asarray(inputs["hidden"])[0])         # [B, D]
    enc = _f32(inputs["encoder_outputs"])              # [L, B, D]
    attn_w = _f32(inputs["attn_w"])                    # [L, 2D]
    attn_b = _f32(inputs["attn_b"]).reshape(1, L)
    comb_w = _f32(inputs["comb_w"])                    # [D, 2D]
    comb_b = _f32(inputs["comb_b"]).reshape(-1)
    w_ih = _f32(inputs["w_ih"])                        # [3D, D]
    w_hh = _f32(inputs["w_hh"])
    b_ih = _f32(inputs["b_ih"]).reshape(-1)
    b_hh = _f32(inputs["b_hh"]).reshape(-1)
    out_w = _f32(inputs["out_w"])                      # [V, D]
    out_b = _f32(inputs["out_b"]).reshape(-1)

    embT = np.ascontiguousarray(embedded.T)            # [D, B]
    h0T = np.ascontiguousarray(h0.T)                   # [D, B]
    attn_wT = np.ascontiguousarray(attn_w.T)           # [2D, L]
    comb_wT = np.ascontiguousarray(comb_w.T)           # [2D, D]
    w_ihT = np.ascontiguousarray(w_ih.T)               # [D, 3D]
    w_hhT = np.ascontiguousarray(w_hh.T)               # [D, 3D]
    out_wT_pad = np.zeros((D, VP), dtype=np.float32)
    out_wT_pad[:, :V] = out_w.T
    out_b_pad = np.full((VP,), NEG_BIG, dtype=np.float32)
    out_b_pad[:V] = out_b

    in_maps = []
    for m in range(NCORES):
        bs = slice(BS * m, BS * (m + 1))
        es = slice(ES * m, ES * (m + 1))
        gsl = np.concatenate([
            np.arange(ES * m, ES * (m + 1)),
            np.arange(D + ES * m, D + ES * (m + 1)),
            np.arange(2 * D + ES * m, 2 * D + ES * (m + 1)),
        ])
        vs = slice(VS * m, VS * (m + 1))
        in_maps.append({
            "embT_sl": np.ascontiguousarray(embT[:, bs]),
            "h0T_sl": np.ascontiguousarray(h0T[:, bs]),
            "embT_full": embT,
            "h0T_full": h0T,
            "h0_sl": np.ascontiguousarray(h0[:, es]),
            "enc_sl": np.ascontiguousarray(enc[:, bs, :]),
            "attn_wT": attn_wT,
            "attn_b": attn_b,
            "comb_wT_sl": np.ascontiguousarray(comb_wT[:, es]),
            "comb_b_sl": np.ascontiguousarray(comb_b[es]).reshape(1, ES),
            "w_ihT_sl": np.ascontiguousarray(w_ihT[:, gsl]),
            "b_ih_sl": np.ascontiguousarray(b_ih[gsl]).reshape(1, G3),
            "w_hhT_sl": np.ascontiguousarray(w_hhT[:, gsl]),
            "b_hh_sl": np.ascontiguousarray(b_hh[gsl]).reshape(1, G3),
            "out_wT_sl": np.ascontiguousarray(out_wT_pad[:, vs]),
            "out_b_sl": np.ascontiguousarray(out_b_pad[vs]).reshape(1, VS),
        })
    return in_maps


def _assemble(results):
    logp = np.concatenate(
        [results[m]["out_logp"] for m in range(NCORES)], axis=1
    )[:, :V]
    h_new = np.concatenate(
        [results[m]["out_hnew"] for m in range(NCORES)], axis=1
    )[None]
    attn = results[0]["out_attn"]
    return (
        np.ascontiguousarray(logp, dtype=np.float32),
        np.ascontiguousarray(h_new, dtype=np.float32),
        np.ascontiguousarray(attn, dtype=np.float32),
    )


def _get_executor():
    """Build (once) a persistent jitted shard_map executable for the program.

    Mirrors concourse.bass2jax.run_bass_via_pjrt's multi-core path, but caches
    the jitted function so repeat kernel() calls skip retracing, and so the
    test harness can time bare executions against device-resident inputs.
    """
    if "exec" in _PROGRAM_CACHE:
        return _PROGRAM_CACHE["exec"]
    nc = _get_program()
    import jax
    from jax.experimental.shard_map import shard_map
    from jax.sharding import Mesh, PartitionSpec

    from concourse import bass2jax

    bass2jax.install_neuronx_cc_hook()
    assert nc.dbg_addr is None
    partition_name = (
        nc.partition_id_tensor.name if nc.partition_id_tensor else None
    )

    in_names, out_names, out_avals = [], [], []
    for alloc in nc.m.functions[0].allocations:
        if not isinstance(alloc, mybir.MemoryLocationSet):
            continue
        name = alloc.memorylocations[0].name
        if alloc.kind == "ExternalInput":
            if name != partition_name:
                in_names.append(name)
        elif alloc.kind == "ExternalOutput":
            out_names.append(name)
            out_avals.append(
                jax.core.ShapedArray(
                    tuple(alloc.tensor_shape), mybir.dt.np(alloc.dtype)
                )
            )
    n_params = len(in_names)
    all_names = tuple(in_names) + tuple(out_names)
    if partition_name is not None:
        all_names = all_names + (partition_name,)

    def _body(*args):
        operands = list(args)
        if partition_name is not None:
            operands.append(bass2jax.partition_id_tensor())
        outs = bass2jax._bass_exec_p.bind(
            *operands,
            out_avals=tuple(out_avals),
            in_names=all_names,
            out_names=tuple(out_names),
            lowering_input_output_aliases=(),
            sim_require_finite=True,
            sim_require_nnan=True,
            nc=nc,
        )
        return tuple(outs)

    devices = jax.devices()[:NCORES]
    assert len(devices) == NCORES
    mesh = Mesh(np.asarray(devices), ("core",))
    n_outs = len(out_names)
    fn = jax.jit(
        shard_map(
            _body,
            mesh=mesh,
            in_specs=(PartitionSpec("core"),) * (n_params + n_outs),
            out_specs=(PartitionSpec("core"),) * n_outs,
            check_rep=False,
        ),
        donate_argnums=tuple(range(n_params, n_params + n_outs)),
        keep_unused=True,
    )
    ex = {
        "fn": fn,
        "mesh": mesh,
        "in_names": in_names,
        "out_names": out_names,
        "out_avals": out_avals,
    }
    _PROGRAM_CACHE["exec"] = ex
    return ex


def _concat_inputs(in_maps, ex):
    return [
        np.concatenate([in_maps[c][n] for c in range(NCORES)], axis=0)
        for n in ex["in_names"]
    ]


def _make_zero_outs(ex):
    return [
        np.zeros((NCORES * a.shape[0], *a.shape[1:]), a.dtype)
        for a in ex["out_avals"]
    ]


def _exec_concat(ex, concat_in, zero_outs):
    out_arrs = ex["fn"](*concat_in, *zero_outs)
    return [
        {
            name: np.asarray(out_arrs[i]).reshape(
                NCORES, *ex["out_avals"][i].shape
            )[c]
            for i, name in enumerate(ex["out_names"])
        }
        for c in range(NCORES)
    ]


def _run(inputs, trace=False):
    in_maps = _prep_in_maps(inputs)
    try:
        ex = _get_executor()
    except Exception:
        nc = _get_program()
        res = run_bass_kernel_spmd(nc, in_maps, core_ids=list(range(NCORES)))
        return _assemble(res.results), res
    results = _exec_concat(ex, _concat_inputs(in_maps, ex), _make_zero_outs(ex))
    return _assemble(results), None


def kernel(**inputs):
    outputs, _ = _run(inputs)
    return outputs
